# revision 10
# baseline (speedup 1.0000x reference)
"""Trainium2 Bass kernel for nn_FLAModel (4-layer GLA transformer + vocab LM head).

Sharding (8 cores): tokens are split into 8 contiguous segments of 512
(core c = batch c//4, sequence quarter c%4).  The backbone runs
sequence-parallel with a tiny per-layer AllGather of GLA chunk-state
summaries; the LM head is vocab-sharded (V padded 50257->51200, 6400/core)
after an AllGather of the final activations.  All layernorm / gate affine
parameters are folded into effective weights on the host; the device
computes in a transposed [feature, token] layout.
"""

import numpy as np
from contextlib import ExitStack

import concourse.bass as bass
import concourse.tile as tile
from concourse import bacc, mybir
from concourse.bass import IndirectOffsetOnAxis
from concourse.bass_utils import run_bass_kernel_spmd

F32 = mybir.dt.float32
I32 = mybir.dt.int32
AF = mybir.ActivationFunctionType
OP = mybir.AluOpType

P = 128
D = 512
DK = 256
DV = 512
H = 8
HK = 32
HV = 64
DH = 2048
GLN = 16.0
EPS = 1e-5
DCH = D // P        # 4
NHC = DH // P       # 16
N_CORES = 8


class Cfg:
    def __init__(self, L=4, TOK=512, VSH=6400, V=50257, gelu=True):
        self.L = L
        self.TOK = TOK            # tokens per core
        self.NCH = TOK // P       # chunks (chunk size == 128)
        self.VSH = VSH            # vocab rows per core (padded)
        self.NVT = VSH // P
        self.V = V
        self.TOKALL = TOK * N_CORES
        self.gelu = gelu          # use ACT Gelu (False -> Identity, for debug)

    def key(self):
        return (self.L, self.TOK, self.VSH, self.V, self.gelu)


def build_program(cfg: Cfg):
    L, TOK, NCH, VSH, NVT, V, TOKALL = (
        cfg.L, cfg.TOK, cfg.NCH, cfg.VSH, cfg.NVT, cfg.V, cfg.TOKALL)
    NTG = TOKALL // 512  # token groups for the head

    nc = bacc.Bacc(None, target_bir_lowering=False)
    nc.num_devices = N_CORES

    # ---------------- DRAM I/O ----------------
    emb_d = nc.dram_tensor("emb", [V, D], F32, kind="ExternalInput")
    tok_d = nc.dram_tensor("tok", [P, NCH], I32, kind="ExternalInput")
    wq_d = nc.dram_tensor("wq", [L, D, DK], F32, kind="ExternalInput")
    wk_d = nc.dram_tensor("wk", [L, D, DK], F32, kind="ExternalInput")
    wv_d = nc.dram_tensor("wv", [L, D, DV], F32, kind="ExternalInput")
    wg_d = nc.dram_tensor("wg", [L, D, DV], F32, kind="ExternalInput")
    wgk1_d = nc.dram_tensor("wgk1", [L, D, P], F32, kind="ExternalInput")
    wgk2_d = nc.dram_tensor("wgk2", [L, P, DK], F32, kind="ExternalInput")
    wo_d = nc.dram_tensor("wo", [L, DV, D], F32, kind="ExternalInput")
    w1_d = nc.dram_tensor("w1", [L, D, DH], F32, kind="ExternalInput")
    w2_d = nc.dram_tensor("w2", [L, DH, D], F32, kind="ExternalInput")
    bq_d = nc.dram_tensor("bq", [L, P, 2], F32, kind="ExternalInput")
    bk_d = nc.dram_tensor("bk", [L, P, 2], F32, kind="ExternalInput")
    bv_d = nc.dram_tensor("bv", [L, 1, DV], F32, kind="ExternalInput")
    bg_d = nc.dram_tensor("bg", [L, 1, DV], F32, kind="ExternalInput")
    bgk_d = nc.dram_tensor("bgk", [L, 1, DK], F32, kind="ExternalInput")
    b1_d = nc.dram_tensor("b1", [L, P, NHC], F32, kind="ExternalInput")
    b2_d = nc.dram_tensor("b2", [L, P, DCH], F32, kind="ExternalInput")
    u_d = nc.dram_tensor("uconst", [P, P], F32, kind="ExternalInput")
    id_d = nc.dram_tensor("ident", [P, P], F32, kind="ExternalInput")
    ones_d = nc.dram_tensor("onescol", [P, 1], F32, kind="ExternalInput")
    mask_d = nc.dram_tensor("maskv", [P, N_CORES], F32, kind="ExternalInput")
    hwt_d = nc.dram_tensor("hwt", [D, VSH], F32, kind="ExternalInput")
    hb_d = nc.dram_tensor("hb", [P, NVT], F32, kind="ExternalInput")
    out_d = nc.dram_tensor("logitsT", [VSH, TOKALL], F32, kind="ExternalOutput")

    rg = [list(range(N_CORES))]

    with tile.TileContext(nc) as tc:
        with ExitStack() as octx:
            dramp = octx.enter_context(
                tc.tile_pool(name="dramp", bufs=1, space="DRAM"))
            cp = octx.enter_context(tc.tile_pool(name="consts", bufs=1))

            U = cp.tile([P, P], F32, name="U")
            nc.sync.dma_start(U[:], u_d[:])
            ident = cp.tile([P, P], F32, name="identsb")
            nc.sync.dma_start(ident[:], id_d[:])
            ones = cp.tile([P, 1], F32, name="onessb")
            nc.sync.dma_start(ones[:], ones_d[:])
            maskv = cp.tile([P, N_CORES], F32, name="masksb")
            nc.sync.dma_start(maskv[:], mask_d[:])
            tok_sb = cp.tile([P, NCH], I32, name="toksb")
            nc.sync.dma_start(tok_sb[:], tok_d[:])
            eps_col = cp.tile([P, 1], F32, name="epscol")
            nc.vector.memset(eps_col[:], EPS)

            ag_in = [dramp.tile([P, 130], F32, name=f"agin{li}") for li in range(L)]
            ag_out = [
                dramp.tile([N_CORES, P, 130], F32, addr_space="Shared",
                           name=f"agout{li}")
                for li in range(L)
            ]
            agx_in = dramp.tile([D, TOK], F32, name="agxin")
            agx_out = dramp.tile([N_CORES, D, TOK], F32, addr_space="Shared",
                                 name="agxout")
            mu_dr = dramp.tile([1, TOK], F32, name="mu_dr")
            rv_dr = dramp.tile([1, TOK], F32, name="rv_dr")

            xp = octx.enter_context(tc.tile_pool(name="xp", bufs=1))
            xT = xp.tile([P, DCH, TOK], F32, name="xT")

            with ExitStack() as lctx:
                wp = lctx.enter_context(tc.tile_pool(name="wts", bufs=1))
                ap_ = lctx.enter_context(tc.tile_pool(name="acts", bufs=1))
                wk_ = lctx.enter_context(tc.tile_pool(name="work", bufs=2))
                pb = lctx.enter_context(
                    tc.tile_pool(name="pbig", bufs=6, space="PSUM"))
                psm = lctx.enter_context(
                    tc.tile_pool(name="psmall", bufs=2, space="PSUM"))

                # ---------------- embedding gather + transpose ----------------
                for j in range(NCH):
                    xg = wk_.tile([P, D], F32, tag="xg", name="xg")
                    nc.gpsimd.indirect_dma_start(
                        out=xg[:], out_offset=None, in_=emb_d[:],
                        in_offset=IndirectOffsetOnAxis(ap=tok_sb[:, j:j + 1], axis=0),
                    )
                    for dc in range(DCH):
                        pt = psm.tile([P, P], F32, tag="sm", name="pt")
                        nc.tensor.transpose(pt[:], xg[:, dc * P:(dc + 1) * P], ident[:])
                        nc.vector.tensor_copy(xT[:, dc, j * P:(j + 1) * P], pt[:])

                def lnt():
                    """Transposed layernorm (pure normalize) of xT -> zT tile.

                    stats tile rows: 0=mu 1=va 2=musq 3=sd 4=rv
                    """
                    ps_s = pb.tile([1, TOK], F32, tag="big", name="ps_s")
                    ps_q = pb.tile([1, TOK], F32, tag="big", name="ps_q")
                    for dc in range(DCH):
                        nc.tensor.matmul(ps_s[:], ones[:, :1], xT[:, dc, :],
                                         start=(dc == 0), stop=(dc == DCH - 1))
                    for dc in range(DCH):
                        xsq = wk_.tile([P, TOK], F32, tag="xsq", name="xsq")
                        nc.vector.tensor_mul(xsq[:], xT[:, dc, :], xT[:, dc, :])
                        nc.tensor.matmul(ps_q[:], ones[:, :1], xsq[:],
                                         start=(dc == 0), stop=(dc == DCH - 1))
                    mu = wk_.tile([1, TOK], F32, tag="mu", name="mu", bufs=1)
                    nc.vector.tensor_scalar_mul(mu[:], ps_s[:], 1.0 / D)
                    va = wk_.tile([1, TOK], F32, tag="va", name="va", bufs=1)
                    nc.vector.tensor_scalar_mul(va[:], ps_q[:], 1.0 / D)
                    musq = wk_.tile([1, TOK], F32, tag="musq", name="musq", bufs=1)
                    nc.vector.tensor_mul(musq[:], mu[:], mu[:])
                    nc.vector.tensor_sub(va[:], va[:], musq[:])
                    nc.vector.tensor_scalar_max(va[:], va[:], 0.0)
                    sd = wk_.tile([1, TOK], F32, tag="sd", name="sd", bufs=1)
                    nc.scalar.activation(sd[:], va[:], AF.Sqrt,
                                         bias=eps_col[:1, :])
                    rv = wk_.tile([1, TOK], F32, tag="rv", name="rv", bufs=1)
                    nc.vector.reciprocal(rv[:], sd[:])
                    nc.sync.dma_start(mu_dr[:], mu[:])
                    nc.sync.dma_start(rv_dr[:], rv[:])
                    murep = wk_.tile([P, TOK], F32, tag="murep", name="murep",
                                     bufs=1)
                    nc.sync.dma_start(murep[:], mu_dr[:].to_broadcast((P, TOK)))
                    rvrep = wk_.tile([P, TOK], F32, tag="rvrep", name="rvrep",
                                     bufs=1)
                    nc.sync.dma_start(rvrep[:], rv_dr[:].to_broadcast((P, TOK)))
                    zT = ap_.tile([P, DCH, TOK], F32, tag="zT", name="zT")
                    for dc in range(DCH):
                        nc.vector.tensor_sub(zT[:, dc, :], xT[:, dc, :], murep[:])
                        nc.vector.tensor_mul(zT[:, dc, :], zT[:, dc, :], rvrep[:])
                    return zT

                # ---------------- transformer layers ----------------
                for li in range(L):
                    wq_sb = wp.tile([P, DCH, DK], F32, tag="wq", name="wq_sb")
                    nc.sync.dma_start(
                        wq_sb[:], wq_d[li].rearrange("(o p) m -> p o m", p=P))
                    wk_sb = wp.tile([P, DCH, DK], F32, tag="wk", name="wk_sb")
                    nc.sync.dma_start(
                        wk_sb[:], wk_d[li].rearrange("(o p) m -> p o m", p=P))
                    wv_sb = wp.tile([P, DCH, DV], F32, tag="wv", name="wv_sb")
                    nc.sync.dma_start(
                        wv_sb[:], wv_d[li].rearrange("(o p) m -> p o m", p=P))
                    wg_sb = wp.tile([P, DCH, DV], F32, tag="wg", name="wg_sb")
                    nc.sync.dma_start(
                        wg_sb[:], wg_d[li].rearrange("(o p) m -> p o m", p=P))
                    wgk1_sb = wp.tile([P, DCH, P], F32, tag="wgk1", name="wgk1_sb")
                    nc.sync.dma_start(
                        wgk1_sb[:], wgk1_d[li].rearrange("(o p) m -> p o m", p=P))
                    wgk2_sb = wp.tile([P, DK], F32, tag="wgk2", name="wgk2_sb")
                    nc.sync.dma_start(wgk2_sb[:], wgk2_d[li])
                    wo_sb = wp.tile([P, DCH, D], F32, tag="wo", name="wo_sb")
                    nc.sync.dma_start(
                        wo_sb[:], wo_d[li].rearrange("(o p) m -> p o m", p=P))
                    bq_sb = wp.tile([P, 2], F32, tag="bq", name="bq_sb")
                    nc.sync.dma_start(bq_sb[:], bq_d[li])
                    bk_sb = wp.tile([P, 2], F32, tag="bk", name="bk_sb")
                    nc.sync.dma_start(bk_sb[:], bk_d[li])
                    b1_sb = wp.tile([P, NHC], F32, tag="b1", name="b1_sb")
                    nc.sync.dma_start(b1_sb[:], b1_d[li])
                    b2_sb = wp.tile([P, DCH], F32, tag="b2", name="b2_sb")
                    nc.sync.dma_start(b2_sb[:], b2_d[li])
                    bv_rep = wp.tile([P, DV], F32, tag="bvr", name="bv_rep")
                    nc.sync.dma_start(bv_rep[:], bv_d[li, 0:1, :].to_broadcast((P, DV)))
                    bg_rep = wp.tile([P, DV], F32, tag="bgr", name="bg_rep")
                    nc.sync.dma_start(bg_rep[:], bg_d[li, 0:1, :].to_broadcast((P, DV)))
                    bgk_rep = wp.tile([P, DK], F32, tag="bgkr", name="bgk_rep")
                    nc.sync.dma_start(
                        bgk_rep[:], bgk_d[li, 0:1, :].to_broadcast((P, DK)))

                    zT = lnt()

                    # ---- projections Q^T, K^T ----
                    QT = ap_.tile([P, 2, TOK], F32, tag="QT", name="QT")
                    KT = ap_.tile([P, 2, TOK], F32, tag="KT", name="KT")
                    for (dst, wsb, bsb) in ((QT, wq_sb, bq_sb), (KT, wk_sb, bk_sb)):
                        for kt in range(2):
                            ps = pb.tile([P, TOK], F32, tag="big", name="psqk")
                            for dc in range(DCH):
                                nc.tensor.matmul(
                                    ps[:], wsb[:, dc, kt * P:(kt + 1) * P],
                                    zT[:, dc, :],
                                    start=(dc == 0), stop=(dc == DCH - 1))
                            nc.vector.tensor_scalar_add(
                                dst[:, kt, :], ps[:], bsb[:, kt:kt + 1])

                    # ---- V and swish-gated G ----
                    V_sb = ap_.tile([P, NCH, DV], F32, tag="Vsb", name="V_sb")
                    sg = ap_.tile([P, NCH, DV], F32, tag="sg", name="sg")
                    for ch in range(NCH):
                        ps = pb.tile([P, DV], F32, tag="big", name="psv")
                        for dc in range(DCH):
                            nc.tensor.matmul(
                                ps[:], zT[:, dc, ch * P:(ch + 1) * P],
                                wv_sb[:, dc, :],
                                start=(dc == 0), stop=(dc == DCH - 1))
                        nc.vector.tensor_add(V_sb[:, ch, :], ps[:], bv_rep[:])
                    for ch in range(NCH):
                        ps = pb.tile([P, DV], F32, tag="big", name="psg")
                        for dc in range(DCH):
                            nc.tensor.matmul(
                                ps[:], zT[:, dc, ch * P:(ch + 1) * P],
                                wg_sb[:, dc, :],
                                start=(dc == 0), stop=(dc == DCH - 1))
                        g0 = wk_.tile([P, DV], F32, tag="g0", name="g0")
                        nc.vector.tensor_add(g0[:], ps[:], bg_rep[:])
                        nc.scalar.activation(sg[:, ch, :], g0[:], AF.Sigmoid)
                        nc.vector.tensor_mul(sg[:, ch, :], sg[:, ch, :], g0[:])

                    # ---- gate path: T1^T -> gkl -> log-sigmoid (LS) ----
                    T1 = ap_.tile([P, TOK], F32, tag="T1", name="T1")
                    psT1 = pb.tile([P, TOK], F32, tag="big", name="psT1")
                    for dc in range(DCH):
                        nc.tensor.matmul(psT1[:], wgk1_sb[:, dc, :], zT[:, dc, :],
                                         start=(dc == 0), stop=(dc == DCH - 1))
                    nc.vector.tensor_copy(T1[:], psT1[:])

                    LS = ap_.tile([P, NCH, DK], F32, tag="tokk", name="LS")
                    for ch in range(NCH):
                        ps2 = pb.tile([P, DK], F32, tag="big", name="psgkl")
                        nc.tensor.matmul(ps2[:], T1[:, ch * P:(ch + 1) * P],
                                         wgk2_sb[:], start=True, stop=True)
                        t0 = wk_.tile([P, DK], F32, tag="t0", name="t0")
                        nc.vector.tensor_add(t0[:], ps2[:], bgk_rep[:])
                        nc.scalar.activation(LS[:, ch, :], t0[:], AF.Sigmoid)
                    for ch in range(NCH):
                        nc.scalar.activation(LS[:, ch, :], LS[:, ch, :], AF.Ln)

                    # ---- cumulative gate sums B^T (<=0), per chunk ----
                    BT = ap_.tile([P, 2, TOK], F32, tag="BT", name="BT")
                    for ch in range(NCH):
                        for kt in range(2):
                            ps3 = psm.tile([P, P], F32, tag="sm", name="psbt")
                            nc.tensor.matmul(
                                ps3[:], LS[:, ch, kt * P:(kt + 1) * P], U[:],
                                start=True, stop=True)
                            nc.vector.tensor_copy(
                                BT[:, kt, ch * P:(ch + 1) * P], ps3[:])

                    # ---- gated Q/K variants ----
                    flat = lambda t: t.rearrange("p a b -> p (a b)")
                    QtT = ap_.tile([P, 2, TOK], F32, tag="QtT", name="QtT")
                    KhT = ap_.tile([P, 2, TOK], F32, tag="KhT", name="KhT")
                    e1 = wk_.tile([P, 2, TOK], F32, tag="esc", name="e1")
                    nc.scalar.activation(flat(e1), flat(BT), AF.Exp, scale=1.0 / GLN)
                    nc.vector.tensor_mul(flat(QtT), flat(QT), flat(e1))
                    e2 = wk_.tile([P, 2, TOK], F32, tag="esc", name="e2")
                    nc.scalar.activation(flat(e2), flat(BT), AF.Exp, scale=-1.0 / GLN)
                    nc.vector.tensor_mul(flat(KhT), flat(KT), flat(e2))

                    # ---- K2 = K * exp((B_last-B_t)/GLN), transposed to [tok,k] ----
                    K2 = ap_.tile([P, NCH, DK], F32, tag="tokk", name="K2")
                    for ch in range(NCH):
                        for kt in range(2):
                            dl = wk_.tile([P, P], F32, tag="dl", name="dl")
                            nc.vector.tensor_scalar(
                                dl[:], BT[:, kt, ch * P:(ch + 1) * P],
                                BT[:, kt, ch * P + P - 1:ch * P + P],
                                None, OP.subtract)
                            nc.scalar.activation(dl[:], dl[:], AF.Exp,
                                                 scale=-1.0 / GLN)
                            nc.vector.tensor_mul(
                                dl[:], dl[:], KT[:, kt, ch * P:(ch + 1) * P])
                            pt = psm.tile([P, P], F32, tag="sm", name="ptk2")
                            nc.tensor.transpose(pt[:], dl[:], ident[:])
                            nc.vector.tensor_copy(
                                K2[:, ch, kt * P:(kt + 1) * P], pt[:])

                    # ---- chunk summaries M_j, zero-seeded prefix Mhat, csum ----
                    Mhat = ap_.tile([P, 2, NCH + 1, HV], F32, tag="Mhat", name="Mhat")
                    csum = ap_.tile([P, 2, NCH + 1], F32, tag="csum", name="csum")
                    nc.vector.memset(Mhat[:, :, 0, :], 0.0)
                    nc.vector.memset(csum[:, :, 0], 0.0)
                    for ch in range(NCH):
                        for g in range(2):
                            mps = psm.tile([P, HV], F32, tag="sm", name="mps")
                            for h4 in range(4):
                                h = g * 4 + h4
                                nc.tensor.matmul(
                                    mps[h4 * HK:(h4 + 1) * HK, :],
                                    K2[:, ch, h * HK:(h + 1) * HK],
                                    V_sb[:, ch, h * HV:(h + 1) * HV],
                                    start=True, stop=True,
                                    tile_position=(0, h4 * HK))
                            dcol = wk_.tile([P, 1], F32, tag="dcol", name="dcol")
                            nc.scalar.activation(
                                dcol[:], BT[:, g, ch * P + P - 1:ch * P + P],
                                AF.Exp, scale=1.0 / GLN)
                            nc.vector.tensor_scalar(
                                Mhat[:, g, ch + 1, :], Mhat[:, g, ch, :],
                                dcol[:, 0:1], None, OP.mult)
                            nc.vector.tensor_add(
                                Mhat[:, g, ch + 1, :], Mhat[:, g, ch + 1, :], mps[:])
                        nc.vector.tensor_add(
                            csum[:, :, ch + 1], csum[:, :, ch],
                            BT[:, :, ch * P + P - 1])

                    # ---- pack + allgather chunk-state summary ----
                    gbuf = wk_.tile([P, 130], F32, tag="gbuf", name="gbuf")
                    nc.vector.tensor_copy(gbuf[:, 0:HV], Mhat[:, 0, NCH, :])
                    nc.vector.tensor_copy(gbuf[:, HV:2 * HV], Mhat[:, 1, NCH, :])
                    nc.scalar.activation(gbuf[:, 128:130], csum[:, :, NCH],
                                         AF.Exp, scale=1.0 / GLN)
                    nc.sync.dma_start(ag_in[li][:], gbuf[:])
                    nc.gpsimd.collective_compute(
                        "AllGather", OP.bypass, replica_groups=rg,
                        ins=[ag_in[li][:]], outs=[ag_out[li][:]])

                    # ---- combine predecessors into S_in (masked, uniform) ----
                    S_in = wk_.tile([P, 2, HV], F32, tag="Sin", name="S_in")
                    nc.vector.memset(flat(S_in), 0.0)
                    for cp_ in range(N_CORES - 1):
                        gin = wk_.tile([P, 130], F32, tag="gin", name="gin")
                        nc.sync.dma_start(gin[:], ag_out[li][cp_])
                        dt_ = wk_.tile([P, 2], F32, tag="dt", name="dt_")
                        nc.vector.tensor_scalar(
                            dt_[:], gin[:, 128:130], 1.0, None, OP.subtract)
                        nc.vector.tensor_scalar(
                            dt_[:], dt_[:], maskv[:, cp_:cp_ + 1], None, OP.mult)
                        nc.vector.tensor_scalar(dt_[:], dt_[:], 1.0, None, OP.add)
                        for g in range(2):
                            nc.vector.tensor_scalar(
                                S_in[:, g, :], S_in[:, g, :], dt_[:, g:g + 1],
                                None, OP.mult)
                            mm = wk_.tile([P, HV], F32, tag="mm", name="mm")
                            nc.vector.tensor_scalar(
                                mm[:], gin[:, g * HV:(g + 1) * HV],
                                maskv[:, cp_:cp_ + 1], None, OP.mult)
                            nc.vector.tensor_add(S_in[:, g, :], S_in[:, g, :], mm[:])

                    # ---- per-chunk start states ----
                    PJ = wk_.tile([P, 2, NCH + 1], F32, tag="PJ", name="PJ")
                    nc.scalar.activation(
                        PJ.rearrange("p a b -> p (a b)"),
                        csum.rearrange("p a b -> p (a b)"), AF.Exp, scale=1.0 / GLN)
                    Sj = ap_.tile([P, 2, NCH, HV], F32, tag="Sj", name="Sj")
                    for ch in range(NCH):
                        for g in range(2):
                            nc.vector.tensor_scalar(
                                Sj[:, g, ch, :], S_in[:, g, :],
                                PJ[:, g, ch:ch + 1], None, OP.mult)
                            nc.vector.tensor_add(
                                Sj[:, g, ch, :], Sj[:, g, ch, :], Mhat[:, g, ch, :])

                    # ---- intra + inter attention per chunk ----
                    SQ = wk_.tile([P, NCH * H], F32, tag="SQ", name="SQ")
                    opss = []
                    for ch in range(NCH):
                        ops = pb.tile([P, DV], F32, tag="big", name="ops")
                        opss.append(ops)
                        for h in range(H):
                            g, hr = h // 4, (h % 4) * HK
                            aps = psm.tile([P, P], F32, tag="sm", name="aps")
                            nc.tensor.matmul(
                                aps[:],
                                KhT[hr:hr + HK, g, ch * P:(ch + 1) * P],
                                QtT[hr:hr + HK, g, ch * P:(ch + 1) * P],
                                start=True, stop=True,
                                tile_position=(hr, 0))
                            am = wk_.tile([P, P], F32, tag="am", name="am")
                            nc.vector.tensor_mul(am[:], aps[:], U[:])
                            nc.tensor.matmul(
                                ops[:, h * HV:(h + 1) * HV], am[:],
                                V_sb[:, ch, h * HV:(h + 1) * HV],
                                start=True, stop=False)
                            nc.tensor.matmul(
                                ops[:, h * HV:(h + 1) * HV],
                                QtT[hr:hr + HK, g, ch * P:(ch + 1) * P],
                                Sj[hr:hr + HK, g, ch, :],
                                start=False, stop=True,
                                tile_position=(hr, 0))
                        osq = wk_.tile([P, DV], F32, tag="osq", name="osq", bufs=1)
                        nc.scalar.square(osq[:], ops[:])
                        nc.vector.tensor_reduce(
                            SQ[:, ch * H:(ch + 1) * H],
                            osq.rearrange("p (h v) -> p h v", v=HV),
                            mybir.AxisListType.X, OP.add)

                    # ---- batched RMS + swish gate (writes into V_sb slot) ----
                    SD2 = wk_.tile([P, NCH * H], F32, tag="SD2", name="SD2")
                    nc.scalar.activation(SD2[:], SQ[:], AF.Sqrt,
                                         scale=1.0 / HV, bias=eps_col[:, :])
                    RV2 = wk_.tile([P, NCH * H], F32, tag="RV2", name="RV2")
                    nc.vector.reciprocal(RV2[:], SD2[:])
                    OG = V_sb  # reuse: V is dead after the O matmuls
                    for ch in range(NCH):
                        for h in range(H):
                            nc.vector.tensor_scalar(
                                OG[:, ch, h * HV:(h + 1) * HV],
                                opss[ch][:, h * HV:(h + 1) * HV],
                                RV2[:, ch * H + h:ch * H + h + 1], None, OP.mult)
                    nc.vector.tensor_mul(flat(OG), flat(OG), flat(sg))

                    # ---- transpose o_gated -> [dv, token] ----
                    OT = ap_.tile([P, DCH, TOK], F32, tag="OT", name="OT")
                    for ch in range(NCH):
                        for vc in range(DCH):
                            pt = psm.tile([P, P], F32, tag="sm", name="ptog")
                            nc.tensor.transpose(
                                pt[:], OG[:, ch, vc * P:(vc + 1) * P], ident[:])
                            nc.vector.tensor_copy(
                                OT[:, vc, ch * P:(ch + 1) * P], pt[:])

                    # ---- attention output projection + residual ----
                    for dc in range(DCH):
                        ps = pb.tile([P, TOK], F32, tag="big", name="psat")
                        for o in range(DCH):
                            nc.tensor.matmul(
                                ps[:], wo_sb[:, o, dc * P:(dc + 1) * P], OT[:, o, :],
                                start=(o == 0), stop=(o == DCH - 1))
                        nc.vector.tensor_add(xT[:, dc, :], xT[:, dc, :], ps[:])

                    # ---- MLP (w1/w2 streamed in quarters) ----
                    z2T = lnt()
                    m2ps = [pb.tile([P, TOK], F32, tag="big", name=f"m2ps{dc}")
                            for dc in range(DCH)]
                    for qtr in range(4):
                        w1q = wp.tile([P, DCH, DH // 4], F32, tag="w1q",
                                      name="w1q", bufs=2)
                        nc.sync.dma_start(
                            w1q[:],
                            w1_d[li].rearrange("(o p) m -> p o m", p=P)
                            [:, :, qtr * (DH // 4):(qtr + 1) * (DH // 4)])
                        w2q = wp.tile([P, NHC // 4, D], F32, tag="w2q",
                                      name="w2q", bufs=2)
                        nc.sync.dma_start(
                            w2q[:],
                            w2_d[li].rearrange("(o p) m -> p o m", p=P)
                            [:, qtr * (NHC // 4):(qtr + 1) * (NHC // 4), :])
                        for hq in range(NHC // 4):
                            hc = qtr * (NHC // 4) + hq
                            ps1 = pb.tile([P, TOK], F32, tag="big", name="ps1")
                            for dc in range(DCH):
                                nc.tensor.matmul(
                                    ps1[:], w1q[:, dc, hq * P:(hq + 1) * P],
                                    z2T[:, dc, :],
                                    start=(dc == 0), stop=(dc == DCH - 1))
                            gel = wk_.tile([P, TOK], F32, tag="gel", name="gel")
                            nc.scalar.activation(
                                gel[:], ps1[:],
                                AF.Gelu if cfg.gelu else AF.Identity,
                                bias=b1_sb[:, hc:hc + 1])
                            for dc in range(DCH):
                                nc.tensor.matmul(
                                    m2ps[dc][:], w2q[:, hq, dc * P:(dc + 1) * P],
                                    gel[:],
                                    start=(hc == 0), stop=(hc == NHC - 1))
                    for dc in range(DCH):
                        tmp = wk_.tile([P, TOK], F32, tag="m2d", name="tmpm2",
                                       bufs=1)
                        nc.vector.tensor_scalar_add(
                            tmp[:], m2ps[dc][:], b2_sb[:, dc:dc + 1])
                        nc.vector.tensor_add(xT[:, dc, :], xT[:, dc, :], tmp[:])

                # ---- final norm + activation allgather ----
                zfT = lnt()
                nc.sync.dma_start(
                    agx_in.rearrange("(o p) t -> p o t", p=P), zfT[:])
                nc.gpsimd.collective_compute(
                    "AllGather", OP.bypass, replica_groups=rg,
                    ins=[agx_in[:]], outs=[agx_out[:]])

            # ---------------- LM head (vocab-sharded) ----------------
            with ExitStack() as hctx:
                hx = hctx.enter_context(tc.tile_pool(name="hx", bufs=1))
                hwp = hctx.enter_context(tc.tile_pool(name="hw", bufs=3))
                hst = hctx.enter_context(tc.tile_pool(name="hst", bufs=2))
                hps = hctx.enter_context(
                    tc.tile_pool(name="hps", bufs=8, space="PSUM"))

                X_sb = hx.tile([P, DCH, TOKALL], F32, name="X_sb")
                for r in range(N_CORES):
                    for dc in range(DCH):
                        nc.sync.dma_start(
                            X_sb[:, dc, r * TOK:(r + 1) * TOK],
                            agx_out[r, dc * P:(dc + 1) * P, :])
                hb_sb = hx.tile([P, NVT], F32, name="hb_sb")
                nc.sync.dma_start(hb_sb[:], hb_d[:])

                hwt_r = hwt_d.rearrange("(o p) v -> p o v", p=P)
                for vt in range(NVT):
                    wt = hwp.tile([P, DCH, P], F32, tag="wt", name="wt")
                    nc.sync.dma_start(wt[:], hwt_r[:, :, vt * P:(vt + 1) * P])
                    ob = hst.tile([P, TOKALL], F32, tag="ob", name="ob")
                    pss = [hps.tile([P, 512], F32, tag="h", name=f"hps{tg}")
                           for tg in range(NTG)]
                    for dc in range(DCH):
                        for tg in range(NTG):
                            nc.tensor.matmul(
                                pss[tg][:], wt[:, dc, :],
                                X_sb[:, dc, tg * 512:(tg + 1) * 512],
                                start=(dc == 0), stop=(dc == DCH - 1))
                    for tg in range(NTG):
                        nc.vector.tensor_scalar_add(
                            ob[:, tg * 512:(tg + 1) * 512], pss[tg][:],
                            hb_sb[:, vt:vt + 1])
                    nc.sync.dma_start(out_d[vt * P:(vt + 1) * P, :], ob[:])

    nc.compile()
    return nc


# ---------------- host side ----------------

def fold_weights(inputs, cfg: Cfg):
    """Fold LN/gate affine params into effective weights; build device layouts."""
    f32 = np.float32
    L = cfg.L
    g = lambda n: np.asarray(inputs[n], f32)
    ln1_w, ln1_b = g("ln1_w"), g("ln1_b")
    ln2_w, ln2_b = g("ln2_w"), g("ln2_b")
    wq, wk, wv, wg = g("wq"), g("wk"), g("wv"), g("wg")
    wgk1, wgk2, bgk, onw = g("wgk1"), g("wgk2"), g("bgk"), g("onw")
    wo, w1, b1, w2, b2 = g("wo"), g("w1"), g("b1"), g("w2"), g("b2")
    scale = HK ** -0.5

    out = {}
    out["wq"] = np.stack([(wq[l] * ln1_w[l][None, :]).T * scale for l in range(L)])
    out["bq"] = np.stack([((wq[l] @ ln1_b[l]) * scale).reshape(2, P).T
                          for l in range(L)])
    out["wk"] = np.stack([(wk[l] * ln1_w[l][None, :]).T for l in range(L)])
    out["bk"] = np.stack([(wk[l] @ ln1_b[l]).reshape(2, P).T for l in range(L)])
    out["wv"] = np.stack([(wv[l] * ln1_w[l][None, :]).T for l in range(L)])
    out["bv"] = np.stack([(wv[l] @ ln1_b[l]).reshape(1, DV) for l in range(L)])
    out["wg"] = np.stack([(wg[l] * ln1_w[l][None, :]).T for l in range(L)])
    out["bg"] = np.stack([(wg[l] @ ln1_b[l]).reshape(1, DV) for l in range(L)])
    wgk1_e = np.zeros((L, D, P), f32)
    wgk2_e = np.zeros((L, P, DK), f32)
    for l in range(L):
        wgk1_e[l, :, :16] = (wgk1[l] * ln1_w[l][None, :]).T
        wgk2_e[l, :16, :] = wgk2[l].T
    out["wgk1"], out["wgk2"] = wgk1_e, wgk2_e
    out["bgk"] = np.stack([
        (bgk[l] + wgk2[l] @ (wgk1[l] @ ln1_b[l])).reshape(1, DK) for l in range(L)])
    out["wo"] = np.stack([(wo[l] * np.tile(onw[l], H)[None, :]).T for l in range(L)])
    out["w1"] = np.stack([(w1[l] * ln2_w[l][None, :]).T for l in range(L)])
    out["b1"] = np.stack([(b1[l] + w1[l] @ ln2_b[l]).reshape(NHC, P).T
                          for l in range(L)])
    out["w2"] = np.stack([w2[l].T for l in range(L)])
    out["b2"] = np.stack([b2[l].reshape(DCH, P).T for l in range(L)])

    lnf_w, lnf_b = g("lnf_w"), g("lnf_b")
    head_w, head_b = g("head_w"), g("head_b")
    VPAD = cfg.VSH * N_CORES
    hw_e = np.zeros((VPAD, D), f32)
    hw_e[:cfg.V] = head_w * lnf_w[None, :]
    hb_e = np.zeros(VPAD, f32)
    hb_e[:cfg.V] = head_b + head_w @ lnf_b
    out["hw_e"], out["hb_e"] = hw_e, hb_e

    out["uconst"] = np.triu(np.ones((P, P), f32))
    out["ident"] = np.eye(P, dtype=f32)
    out["onescol"] = np.ones((P, 1), f32)
    out["emb"] = np.ascontiguousarray(g("embed"))
    return out


def make_in_maps(inputs, folded, cfg: Cfg):
    f32 = np.float32
    tokens = np.asarray(inputs["tokens"]).reshape(-1).astype(np.int32)
    maps = []
    for c in range(N_CORES):
        m = {}
        for n in ("wq", "wk", "wv", "wg", "wgk1", "wgk2", "wo", "w1", "w2",
                  "bq", "bk", "bv", "bg", "bgk", "b1", "b2",
                  "uconst", "ident", "onescol", "emb"):
            m[n] = folded[n]
        tok_c = tokens[c * cfg.TOK:(c + 1) * cfg.TOK].reshape(cfg.NCH, P).T
        m["tok"] = np.ascontiguousarray(tok_c)
        mask = np.zeros(N_CORES, f32)
        for cp_ in range(N_CORES):
            if cp_ // 4 == c // 4 and cp_ < c:
                mask[cp_] = 1.0
        m["maskv"] = np.broadcast_to(mask, (P, N_CORES)).copy()
        hw_c = folded["hw_e"][c * cfg.VSH:(c + 1) * cfg.VSH]
        m["hwt"] = np.ascontiguousarray(hw_c.T)
        hb_c = folded["hb_e"][c * cfg.VSH:(c + 1) * cfg.VSH]
        m["hb"] = np.ascontiguousarray(hb_c.reshape(cfg.NVT, P).T)
        maps.append(m)
    return maps


_CACHE = {}


def get_program(cfg: Cfg):
    k = cfg.key()
    if k not in _CACHE:
        _CACHE[k] = build_program(cfg)
    return _CACHE[k]


def kernel(**inputs) -> np.ndarray:
    cfg = Cfg()
    nc = get_program(cfg)
    folded = fold_weights(inputs, cfg)
    in_maps = make_in_maps(inputs, folded, cfg)
    res = run_bass_kernel_spmd(nc, in_maps, core_ids=list(range(N_CORES)))
    logitsT = np.concatenate(
        [res.results[c]["logitsT"] for c in range(N_CORES)], axis=0)[:cfg.V]
    B, S = np.asarray(inputs["tokens"]).shape
    return np.ascontiguousarray(logitsT.T).reshape(B, S, cfg.V)
